# revision 1
# baseline (speedup 1.0000x reference)
"""Trainium2 Bass kernel for the binarized CNN:
conv3x3(sign weights) -> BN -> ternary hardtanh -> maxpool4 -> linear(sign weights)

Strategy (pure data parallel over batch, 8 cores x 512 samples):
  - Host folds BN scale into a Toeplitz conv-weight matrix W[115, 1152]
    (rows = 3x38 patch + ones-row carrying the affine bias), so the conv
    for one output row h is ONE K=115 matmul: z[b, (c,w)] = imc_h.T @ W.
  - maxpool commutes with the monotone affine+ternary (bn_gamma > 0), so we
    pool RAW conv outputs: w-pool via strided reduce_max straight from PSUM,
    h-pool via tensor_max over the 4 row tiles.
  - ternary(y) = (y > 0.5) - (y < -0.5)  (exactly clip+round-half-even).
  - FC: PE-transpose ternary tiles to [feature, batch], 9 accumulating
    matmuls against host-permuted sign(fc_w), add bias, transpose out.
"""

import numpy as np
from contextlib import ExitStack

import concourse.bass as bass
import concourse.tile as tile
from concourse import bacc, mybir
from concourse.bass_utils import run_bass_kernel_spmd

F32 = mybir.dt.float32
F32R = mybir.dt.float32r
BF16 = mybir.dt.bfloat16
U16 = mybir.dt.uint16
ALU = mybir.AluOpType

# Dedupe identical LDWEIGHTS (bass emits one per matmul; fp32 has no
# background weight buffer so redundant loads serialize on the PE).
import os as _os
if _os.environ.get("KLDW", "1") == "1":
    from concourse import bass_utils as _bu
    if not getattr(_bu, "_ldw_patched", False):
        _orig_gwa = _bu.get_walrus_args

        def _gwa(*a, **k):
            return [x if x != "--enable-ldw-opt=false" else "--enable-ldw-opt=true"
                    for x in _orig_gwa(*a, **k)]

        _bu.get_walrus_args = _gwa
        _bu._ldw_patched = True

NCORES = 8
BFULL = 4096
B = BFULL // NCORES          # 512 per core
P = 128
BT = B // P                  # 4 batch tiles
H, W = 14, 38
HO, WO = 12, 36
C = 32
KP = 3 * W + 1               # 115 = 114 patch rows + ones row
NF = C * WO                  # 1152 conv outputs per (b, h)
CW3 = C * (WO // 4)          # 288 after w-pool
EPS = 1e-5
NOUT = 10


def _host_prep(conv_w, conv_b, bn_gamma, bn_beta, bn_mean, bn_var, fc_w, fc_b):
    inv = (bn_gamma / np.sqrt(bn_var + EPS)).astype(np.float32)
    tb = ((conv_b - bn_mean) * inv + bn_beta).astype(np.float32)
    sw = np.sign(conv_w[:, 0]).astype(np.float32)          # [32, 3, 3]

    wt = np.zeros((KP, NF), np.float32)
    for c in range(C):
        for w in range(WO):
            n = c * WO + w
            for i in range(3):
                for j in range(3):
                    wt[i * W + w + j, n] = sw[c, i, j] * inv[c]
            wt[114, n] = tb[c]

    sf = np.sign(fc_w).astype(np.float32)                  # [10, 864]
    sfc = np.zeros((P, 9 * NOUT), np.float32)              # cast to bf16 at return
    for jj in range(9):
        h3, ch = jj // 3, jj % 3
        kj = 32 if ch == 2 else 128
        for r in range(kj):
            rg = ch * 128 + r                              # index into (c, w3)
            c, w3 = rg // 9, rg % 9
            f = c * 27 + h3 * 9 + w3                       # reference flatten order
            sfc[r, jj * NOUT:(jj + 1) * NOUT] = sf[:, f]

    import ml_dtypes
    fcb = fc_b.astype(np.float32).reshape(NOUT, 1)
    eye = np.eye(P, dtype=np.float32)
    ones = np.ones((1, BT * HO * P), np.float32)
    return wt, sfc.astype(ml_dtypes.bfloat16), fcb, eye, ones


def _build():
    nc = bacc.Bacc("TRN2", target_bir_lowering=False, debug=False,
                   num_devices=NCORES)
    x_d = nc.dram_tensor("x", [B, 5 * P], F32, kind="ExternalInput").ap()
    wt_d = nc.dram_tensor("wt", [KP, NF], F32, kind="ExternalInput").ap()
    sfc_d = nc.dram_tensor("sfc", [P, 9 * NOUT], BF16, kind="ExternalInput").ap()
    fcb_d = nc.dram_tensor("fcb", [NOUT, 1], F32, kind="ExternalInput").ap()
    id_d = nc.dram_tensor("ident", [P, P], F32, kind="ExternalInput").ap()
    on_d = nc.dram_tensor("ones", [1, BT * HO * P], F32, kind="ExternalInput").ap()
    out_d = nc.dram_tensor("out", [B, NOUT], F32, kind="ExternalOutput").ap()

    with tile.TileContext(nc) as tc, ExitStack() as ctx:
        const = ctx.enter_context(tc.tile_pool(name="const", bufs=1))
        xbp = ctx.enter_context(tc.tile_pool(name="xb", bufs=1))
        xtp = ctx.enter_context(tc.tile_pool(name="xt", bufs=1))
        imcp = ctx.enter_context(tc.tile_pool(name="imc", bufs=1))
        up = ctx.enter_context(tc.tile_pool(name="u", bufs=6))
        yp = ctx.enter_context(tc.tile_pool(name="y", bufs=3))
        gp = ctx.enter_context(tc.tile_pool(name="g", bufs=6))
        ttp = ctx.enter_context(tc.tile_pool(name="tt", bufs=1))

        wt = const.tile([KP, NF], F32, tag="wt")
        nc.scalar.dma_start(wt[:], wt_d)
        sfc = const.tile([P, 9 * NOUT], BF16, tag="sfc")
        nc.scalar.dma_start(sfc[:], sfc_d)
        fcb = const.tile([NOUT, 1], F32, tag="fcb")
        nc.scalar.dma_start(fcb[:], fcb_d)
        idm = const.tile([P, P], F32, tag="idm")
        nc.scalar.dma_start(idm[:], id_d)

        tT = [ttp.tile([P, B], BF16, tag=f"tT{j}", name=f"tT{j}") for j in range(9)]

        im = imcp.tile([KP, BT * HO * P], F32, tag="imc")
        xbs, xts = {}, {}

        def stage_load(bt):
            xb = xbp.tile([P, 5 * P], F32, tag=f"xb{bt}", name=f"xb{bt}")
            nc.sync.dma_start(xb[:, :], x_d[bt * P:(bt + 1) * P, :])
            xbs[bt] = xb

        def stage_transpose(bt, pp):
            xt = xtp.tile([P, 5 * P], F32, tag=f"xt{bt}", name=f"xt{bt}")
            for t5 in range(5):
                ft = min(P, H * W - t5 * P)
                pt = pp.tile([P, P], F32, tag="pt", name="pt")
                nc.tensor.transpose(pt[:, :],
                                    xbs[bt][:, t5 * P:(t5 + 1) * P], idm[:])
                if t5 % 2 == 0:
                    nc.scalar.copy(xt[0:ft, t5 * P:(t5 + 1) * P],
                                   pt[0:ft, 0:P])
                else:
                    nc.vector.tensor_copy(xt[0:ft, t5 * P:(t5 + 1) * P],
                                          pt[0:ft, 0:P])
            xts[bt] = xt

        def stage_im2col(bt):
            for h in range(HO):
                k = bt * HO + h
                eng = nc.sync if k % 2 == 0 else nc.scalar
                r0 = 38 * h
                a = r0
                while a < r0 + 114:
                    t5 = a // P
                    b_ = min(r0 + 114, (t5 + 1) * P)
                    eng.dma_start(
                        im[a - r0:b_ - r0, k * P:(k + 1) * P],
                        xts[bt][a - t5 * P:b_ - t5 * P,
                                t5 * P:(t5 + 1) * P])
                    a = b_

        def conv_tile(bt, zp, fc_hook=None):
            us = {}
            for h in range(HO):
                k = bt * HO + h
                z = zp.tile([P, NF], F32, tag="z", name="z")
                for n0, n1 in ((0, 512), (512, 1024), (1024, NF)):
                    nc.tensor.matmul(z[:, n0:n1],
                                     lhsT=im[:, k * P:(k + 1) * P],
                                     rhs=wt[:, n0:n1],
                                     start=True, stop=True)
                u = up.tile([P, CW3], F32, tag="u", name="u")
                nc.vector.reduce_max(
                    u[:], z[:].rearrange("p (cw ww) -> p cw ww", ww=4),
                    axis=mybir.AxisListType.X)
                us[h] = u

                if h % 4 == 3:
                    h3 = h // 4
                    y01 = yp.tile([P, CW3], F32, tag="ya", name="ya")
                    nc.vector.tensor_max(y01[:], us[4 * h3][:],
                                         us[4 * h3 + 1][:])
                    y23 = yp.tile([P, CW3], F32, tag="yb", name="yb")
                    nc.vector.tensor_max(y23[:], us[4 * h3 + 2][:],
                                         us[4 * h3 + 3][:])
                    y = yp.tile([P, CW3], F32, tag="yc", name="yc")
                    nc.vector.tensor_max(y[:], y01[:], y23[:])
                    g = gp.tile([P, CW3], F32, tag="gg", name="gg")
                    nc.vector.tensor_scalar(g[:], y[:], 0.5, None, ALU.is_gt)
                    l = gp.tile([P, CW3], F32, tag="ll", name="ll")
                    nc.vector.tensor_scalar(l[:], y[:], -0.5, None, ALU.is_lt)
                    t_ = gp.tile([P, 3 * P], BF16, tag="t_", name="t_")
                    nc.gpsimd.memset(t_[:, CW3:3 * P], 0.0)
                    nc.vector.tensor_sub(t_[:, 0:CW3], g[:], l[:])
                    for ch in range(3):
                        eng = nc.scalar if ch == 1 else nc.sync
                        eng.dma_start_transpose(
                            tT[h3 * 3 + ch][:, bt * P:(bt + 1) * P],
                            t_[:, ch * P:(ch + 1) * P])
                    if fc_hook is not None:
                        fc_hook(h3)

        with tc.tile_pool(name="zp", bufs=2, space="PSUM") as zp:
            nc.gpsimd.dma_start(im[114:115, :], on_d)
            with tc.tile_pool(name="pp", bufs=2, space="PSUM") as pp:
                for bt in range(BT):
                    stage_load(bt)
                for bt in range(BT):
                    stage_transpose(bt, pp)
                for bt in range(BT):
                    stage_im2col(bt)
                conv_tile(0, zp)

            with tc.tile_pool(name="fcp", bufs=1, space="PSUM") as fcp:
                acc = fcp.tile([NOUT, B], F32, tag="acc")
                conv_tile(1, zp)
                conv_tile(2, zp)

                def fc_hook(h3):
                    for j in (3 * h3, 3 * h3 + 1, 3 * h3 + 2):
                        kj = 32 if j % 3 == 2 else 128
                        nc.tensor.matmul(acc[:, :],
                                         lhsT=sfc[0:kj,
                                                  j * NOUT:(j + 1) * NOUT],
                                         rhs=tT[j][0:kj, :],
                                         start=(j == 0), stop=(j == 8))

                conv_tile(3, zp, fc_hook=fc_hook)

                ob = const.tile([P, B], F32, tag="ob")
                nc.vector.memset(ob[:], 0.0)
                nc.scalar.activation(ob[0:NOUT, :], acc[:],
                                     mybir.ActivationFunctionType.Identity,
                                     bias=fcb[0:NOUT, 0:1], scale=1.0)

        with tc.tile_pool(name="otp", bufs=2, space="PSUM") as otp:
            for bt in range(BT):
                po = otp.tile([P, P], F32, tag="po", name="po")
                nc.tensor.transpose(po[:, :],
                                    ob[:, bt * P:(bt + 1) * P],
                                    idm[:])
                os_ = const.tile([P, NOUT], F32, tag=f"os{bt}", name=f"os{bt}")
                nc.scalar.copy(os_[:], po[0:P, 0:NOUT])
                nc.sync.dma_start(out_d[bt * P:(bt + 1) * P, :], os_[:])

    nc.compile()
    return nc


_NC_CACHE = None


def kernel(x, conv_w, conv_b, bn_gamma, bn_beta, bn_mean, bn_var, fc_w, fc_b):
    global _NC_CACHE
    x = np.asarray(x, np.float32).reshape(BFULL, H * W)
    x = np.pad(x, ((0, 0), (0, 5 * P - H * W)))
    wt, sfc, fcb, eye, ones = _host_prep(
        np.asarray(conv_w, np.float32), np.asarray(conv_b, np.float32),
        np.asarray(bn_gamma, np.float32), np.asarray(bn_beta, np.float32),
        np.asarray(bn_mean, np.float32), np.asarray(bn_var, np.float32),
        np.asarray(fc_w, np.float32), np.asarray(fc_b, np.float32))

    if _NC_CACHE is None:
        _NC_CACHE = _build()
    nc = _NC_CACHE

    in_maps = [
        dict(x=np.ascontiguousarray(x[i * B:(i + 1) * B]),
             wt=wt, sfc=sfc, fcb=fcb, ident=eye, ones=ones)
        for i in range(NCORES)
    ]
    res = run_bass_kernel_spmd(nc, in_maps, core_ids=list(range(NCORES)))
    out = np.concatenate([res.results[i]["out"] for i in range(NCORES)], axis=0)
    return out.astype(np.float32)

